# revision 4
# baseline (speedup 1.0000x reference)
"""Trainium2 Bass kernel for nn_ConditionalRandomField_52913997087452.

Computes sum_b [ gold_path_score(b) - log Z(b) ] for a linear-chain CRF with
B=128, L=1024, T=128, mask all-ones.

Strategy (data-parallel over batch, 16 per core x 8 cores), bidirectional:
  - The per-core serial bottleneck is the alpha recurrence's cross-engine
    latency (PE matmul visibility + DVE PSUM-read multiply), ~535 ns/step.
    Instead of one 1023-step forward chain, run TWO independent chains
    concurrently and meet in the middle:
        forward:  pi_t = f_t * (Ehat^T pi_{t-1}),  t = 1..MID
        backward: c_t  = f_t * (Ehat   c_{t+1}),   t = 1022..MID+1
    with f_t = exp(logits_t) (start folded into t=0, end into t=L-1),
    Ehat = exp(transitions - ghat).  Then per batch column
        Z * e^{-(L-1) ghat} / (renorm scales) = sum_j (Ehat^T pi_MID)[j] * c_{MID+1}[j].
  - Emissions F are DMA'd in a host-pretransposed [T, B, L] layout (no PE
    transposes on device) and exponentiated by the Act engine. The first
    pieces of chunks 0 and 7 are sliced fine so both chains start early.
  - Periodic renormalization: a ones-vector matmul on the PE computes
    S[b] = sum_i v[i,b]; 1/S is folded into a later F column off the critical
    path (Act copy + DVE reciprocal + GPSIMD broadcast/multiply), and S is
    streamed out so the host adds back sum_k log S_k. Fwd/bwd renorm steps
    are staggered so the DVE never sees two renorms at once.
  - The meet product u = (Ehat^T pi_MID) * c_{MID+1} is DMA'd out; the host
    does the final tag-sum and log. The gold-path numerator is a tiny
    gather-and-sum done on the host.

The kernel builder is cached at module level so repeated kernel() calls
reuse the compiled program.
"""
import sys

if "/opt/trn_rl_repo" not in sys.path:
    sys.path.insert(0, "/opt/trn_rl_repo")

import numpy as np

import concourse.bacc as bacc
import concourse.tile as tile
from concourse import mybir
from concourse.bass_utils import run_bass_kernel_spmd

B = 128
L = 1024
T = 128
NCORES = 8
BPC = B // NCORES       # batch per core
NCH = L // 128          # 128-column F chunks
APPLY_DELAY = 8         # fold 1/S into F column +/- APPLY_DELAY steps ahead
MID = 512               # fwd produces pi_1..pi_MID; bwd produces c_1022..c_{MID+1}
NSTEP_F = MID           # 512 fwd multiply steps
NSTEP_B = L - 2 - MID   # 510 bwd multiply steps
FWD_REN = [64, 192, 320, 448]    # renorm after producing pi_t at these t
BWD_REN = [896, 768, 640, 544]   # renorm after producing c_t at these t
NREN = len(FWD_REN) + len(BWD_REN)


def _build():
    nc = bacc.Bacc("TRN2", target_bir_lowering=False)
    # host-pretransposed emissions: [tag, batch, time]
    lg = nc.dram_tensor("lg", [T, BPC, L], mybir.dt.float32, kind="ExternalInput")
    eh = nc.dram_tensor("eh", [T, T], mybir.dt.float32, kind="ExternalInput")
    ehT = nc.dram_tensor("ehT", [T, T], mybir.dt.float32, kind="ExternalInput")
    u_out = nc.dram_tensor("u", [T, BPC], mybir.dt.float32, kind="ExternalOutput")
    s_out = nc.dram_tensor("s", [1, NREN * BPC], mybir.dt.float32,
                           kind="ExternalOutput")

    with tile.TileContext(nc) as tc:
        with (
            tc.tile_pool(name="consts", bufs=1) as consts,
            tc.tile_pool(name="fpool", bufs=1) as fpool,
            tc.tile_pool(name="rawpool", bufs=3) as rawpool,
            tc.tile_pool(name="pipool", bufs=4) as pipool,
            tc.tile_pool(name="cipool", bufs=4) as cipool,
            tc.tile_pool(name="mmpsA", bufs=2, space="PSUM") as mmpsA,
            tc.tile_pool(name="mmpsB", bufs=2, space="PSUM") as mmpsB,
            tc.tile_pool(name="spsum", bufs=2, space="PSUM") as spsum,
            tc.tile_pool(name="rpool", bufs=2) as rpool,
        ):
            # ---- emissions F: 8 chunk tiles [T, BPC, 128] f32 ----
            F = []
            for c in range(NCH):
                fc = fpool.tile([T, BPC, 128], mybir.dt.float32, tag=f"F{c}",
                                name=f"F{c}")
                F.append(fc)

            def fcol(t):
                return F[t // 128][:, :, t % 128]

            raw0 = rawpool.tile([T, BPC, 128], mybir.dt.float32, tag="raw",
                                name="raw0")
            raw7 = rawpool.tile([T, BPC, 128], mybir.dt.float32, tag="raw",
                                name="raw7")

            def dma_piece(rawt, c, lo, hi):
                nc.sync.dma_start(out=rawt[:, :, lo:hi],
                                  in_=lg[:, :, c * 128 + lo:c * 128 + hi])

            def exp_piece(rawt, c, lo, hi):
                nc.scalar.activation(
                    out=F[c][:, :, lo:hi], in_=rawt[:, :, lo:hi],
                    func=mybir.ActivationFunctionType.Exp,
                )

            # fine-sliced first pieces so both chains launch early; the DMA
            # emission order below is the HWDGE grant order
            dma_piece(raw0, 0, 0, 16)
            eh_t = consts.tile([T, T], mybir.dt.float32)
            nc.sync.dma_start(out=eh_t[:], in_=eh[:, :])
            dma_piece(raw7, 7, 112, 128)
            ehT_t = consts.tile([T, T], mybir.dt.float32)
            nc.sync.dma_start(out=ehT_t[:], in_=ehT[:, :])
            exp_piece(raw0, 0, 0, 16)
            exp_piece(raw7, 7, 112, 128)
            dma_piece(raw0, 0, 16, 64)
            dma_piece(raw7, 7, 64, 112)
            exp_piece(raw0, 0, 16, 64)
            exp_piece(raw7, 7, 64, 112)
            dma_piece(raw0, 0, 64, 128)
            dma_piece(raw7, 7, 0, 64)
            exp_piece(raw0, 0, 64, 128)
            exp_piece(raw7, 7, 0, 64)

            ones_t = consts.tile([T, 1], mybir.dt.float32)
            nc.vector.memset(ones_t[:], 1.0)
            sacc = consts.tile([1, NREN * BPC], mybir.dt.float32)

            for c in (1, 6, 2, 5, 3, 4):
                rawc = rawpool.tile([T, BPC, 128], mybir.dt.float32, tag="raw",
                                    name=f"raw{c}")
                nc.sync.dma_start(out=rawc[:],
                                  in_=lg[:, :, c * 128:(c + 1) * 128])
                halves = (0, 1) if c < 4 else (1, 0)
                for h in halves:
                    exp_piece(rawc, c, h * 64, (h + 1) * 64)

            def renorm(v_ap, slot, fold_t):
                sp = spsum.tile([1, BPC], mybir.dt.float32, tag="sp", name="sp")
                nc.tensor.matmul(sp[:], ones_t[:], v_ap)
                nc.scalar.activation(
                    out=sacc[:, slot * BPC:(slot + 1) * BPC], in_=sp[:],
                    func=mybir.ActivationFunctionType.Copy,
                )
                rec = rpool.tile([1, BPC], mybir.dt.float32, tag="rec",
                                 name="rec")
                nc.vector.reciprocal(out=rec[:], in_=sp[:])
                rb = rpool.tile([T, BPC], mybir.dt.float32, tag="rb", name="rb")
                nc.gpsimd.partition_broadcast(rb[:], rec[:])
                # fold 1/S into a future F column, off the critical path
                nc.gpsimd.tensor_tensor(out=fcol(fold_t), in0=fcol(fold_t),
                                        in1=rb[:], op=mybir.AluOpType.mult)

            # ---- bidirectional recurrence, interleaved emission ----
            pi_ap = fcol(0)       # pi_0 = exp(lg_0 + start)
            ci_ap = fcol(L - 1)   # c_{L-1} = exp(lg_{L-1} + end)
            for k in range(NSTEP_F):
                tf = k + 1
                psf = mmpsA.tile([T, BPC], mybir.dt.float32, tag="psf",
                                 name="psf")
                nc.tensor.matmul(psf[:], eh_t[:], pi_ap)
                if k < NSTEP_B:
                    tb = L - 2 - k
                    psb = mmpsB.tile([T, BPC], mybir.dt.float32, tag="psb",
                                     name="psb")
                    nc.tensor.matmul(psb[:], ehT_t[:], ci_ap)
                npi = pipool.tile([T, BPC], mybir.dt.float32, tag="pi",
                                  name="pi")
                nc.vector.tensor_tensor(out=npi[:], in0=psf[:], in1=fcol(tf),
                                        op=mybir.AluOpType.mult)
                pi_ap = npi[:]
                if k < NSTEP_B:
                    nci = cipool.tile([T, BPC], mybir.dt.float32, tag="ci",
                                      name="ci")
                    nc.vector.tensor_tensor(out=nci[:], in0=psb[:],
                                            in1=fcol(tb),
                                            op=mybir.AluOpType.mult)
                    ci_ap = nci[:]

                if tf in FWD_REN:
                    renorm(pi_ap, FWD_REN.index(tf), tf + APPLY_DELAY)
                if k < NSTEP_B and tb in BWD_REN:
                    renorm(ci_ap, 4 + BWD_REN.index(tb), tb - APPLY_DELAY)

            # ---- meet in the middle: u = (Ehat^T pi_MID) * c_{MID+1} ----
            psq = mmpsA.tile([T, BPC], mybir.dt.float32, tag="psf", name="psq")
            nc.tensor.matmul(psq[:], eh_t[:], pi_ap)
            u = pipool.tile([T, BPC], mybir.dt.float32, tag="pi", name="u")
            nc.vector.tensor_tensor(out=u[:], in0=psq[:], in1=ci_ap,
                                    op=mybir.AluOpType.mult)
            nc.sync.dma_start(out=s_out[:, :], in_=sacc[:])
            nc.sync.dma_start(out=u_out[:, :], in_=u[:])

    nc.compile()
    return nc


_NC_CACHE = None


def _get_nc():
    global _NC_CACHE
    if _NC_CACHE is None:
        _NC_CACHE = _build()
    return _NC_CACHE


def kernel(inputs, tags, mask, transitions, start_transitions, end_transitions):
    logits = np.ascontiguousarray(inputs, dtype=np.float32)
    trans = np.asarray(transitions, dtype=np.float32)
    start_t = np.asarray(start_transitions, dtype=np.float32)
    end_t = np.asarray(end_transitions, dtype=np.float32)
    tags_i = np.asarray(tags).astype(np.int64, copy=False)
    maskf = np.asarray(mask).astype(np.float64)

    # ---------- device part: log-partition via bidirectional scaled pass ----
    lg = logits.copy()
    lg[:, 0, :] += start_t[None, :]
    lg[:, -1, :] += end_t[None, :]
    E = np.exp(trans.astype(np.float64))
    ghat = float(np.log(T * E.mean()))
    eh = (E * np.exp(-ghat)).astype(np.float32)
    ehT = np.ascontiguousarray(eh.T)
    # [NCORES, T, BPC, L]: tag-major per core so device DMAs need no transpose
    lgT = np.ascontiguousarray(
        lg.reshape(NCORES, BPC, L, T).transpose(0, 3, 1, 2))

    nc = _get_nc()
    in_maps = []
    for c in range(NCORES):
        in_maps.append({
            "lg": lgT[c],
            "eh": eh,
            "ehT": ehT,
        })
    res = run_bass_kernel_spmd(nc, in_maps, core_ids=list(range(NCORES)))

    u = np.stack([res.results[c]["u"] for c in range(NCORES)])     # (8, T, BPC)
    s = np.stack([res.results[c]["s"] for c in range(NCORES)])     # (8, 1, NREN*BPC)
    w = u.astype(np.float64).sum(axis=1)                           # (8, BPC)
    logZ = np.log(w.reshape(NCORES * BPC))
    srs = s.reshape(NCORES, NREN, BPC).astype(np.float64)
    logZ += np.log(srs).sum(axis=1).reshape(-1)
    logZ += (L - 1) * ghat

    # ---------- host part: gold-path numerator (tiny gathers) ----------
    lf64 = logits.astype(np.float64)
    emit = np.take_along_axis(lf64, tags_i[..., None], axis=2)[..., 0]   # (B, L)
    trans_sc = trans.astype(np.float64)[tags_i[:, :-1], tags_i[:, 1:]]   # (B, L-1)
    score = start_t.astype(np.float64)[tags_i[:, 0]]
    score = score + (trans_sc * maskf[:, 1:]).sum(axis=1)
    score = score + (emit[:, :-1] * maskf[:, :-1]).sum(axis=1)
    last_idx = maskf.astype(np.int64).sum(axis=1) - 1
    last_tags = np.take_along_axis(tags_i, last_idx[:, None], axis=1)[:, 0]
    last_input_score = lf64[np.arange(B), -1, last_tags]
    score = score + end_t.astype(np.float64)[last_tags] + last_input_score * maskf[:, -1]

    return np.float32(np.sum(score - logZ))


# revision 7
# speedup vs baseline: 1.0037x; 1.0037x over previous
"""Trainium2 Bass kernel for nn_ConditionalRandomField_52913997087452.

Computes sum_b [ gold_path_score(b) - log Z(b) ] for a linear-chain CRF with
B=128, L=1024, T=128, mask all-ones.

Strategy (data-parallel over batch, 16 per core x 8 cores), bidirectional:
  - The per-core serial bottleneck is the alpha recurrence's cross-engine
    latency (PE matmul visibility + DVE PSUM-read multiply), ~535 ns/step.
    Instead of one 1023-step forward chain, run TWO independent chains
    concurrently and meet in the middle:
        forward:  pi_t = f_t * (Ehat^T pi_{t-1}),  t = 1..MID
        backward: c_t  = f_t * (Ehat   c_{t+1}),   t = 1022..MID+1
    with f_t = exp(logits_t) (start folded into t=0, end into t=L-1),
    Ehat = exp(transitions - ghat).  Then per batch column
        Z * e^{-(L-1) ghat} / (renorm scales) = sum_j (Ehat^T pi_MID)[j] * c_{MID+1}[j].
  - Emissions F are DMA'd in a host-pretransposed [T, B, L] layout (no PE
    transposes on device) and exponentiated by the Act engine. The first
    pieces of chunks 0 and 7 are sliced fine so both chains start early.
  - Periodic renormalization: a ones-vector matmul on the PE computes
    S[b] = sum_i v[i,b]; 1/S is folded into a later F column off the critical
    path (Act copy + DVE reciprocal + GPSIMD broadcast/multiply), and S is
    streamed out so the host adds back sum_k log S_k. Fwd/bwd renorm steps
    are staggered so the DVE never sees two renorms at once.
  - The meet product u = (Ehat^T pi_MID) * c_{MID+1} is DMA'd out; the host
    does the final tag-sum and log. The gold-path numerator is a tiny
    gather-and-sum done on the host.

The kernel builder is cached at module level so repeated kernel() calls
reuse the compiled program.
"""
import sys

if "/opt/trn_rl_repo" not in sys.path:
    sys.path.insert(0, "/opt/trn_rl_repo")

import numpy as np

import concourse.bacc as bacc
import concourse.tile as tile
from concourse import mybir
from concourse.bass_utils import run_bass_kernel_spmd

B = 128
L = 1024
T = 128
NCORES = 8
BPC = B // NCORES       # batch per core
NCH = L // 128          # 128-column F chunks
APPLY_DELAY = 8         # fold 1/S into F column +/- APPLY_DELAY steps ahead
MID = 512               # fwd produces pi_1..pi_MID; bwd produces c_1022..c_{MID+1}
NSTEP_F = MID           # 512 fwd multiply steps
NSTEP_B = L - 2 - MID   # 510 bwd multiply steps
FWD_REN = [64, 192, 320, 448]    # renorm after producing pi_t at these t
BWD_REN = [896, 768, 640, 544]   # renorm after producing c_t at these t
NREN = len(FWD_REN) + len(BWD_REN)


def _build():
    nc = bacc.Bacc("TRN2", target_bir_lowering=False)
    # host-pretransposed emissions: [tag, batch, time]
    lg = nc.dram_tensor("lg", [T, BPC, L], mybir.dt.float32, kind="ExternalInput")
    eh = nc.dram_tensor("eh", [T, T], mybir.dt.float32, kind="ExternalInput")
    ehT = nc.dram_tensor("ehT", [T, T], mybir.dt.float32, kind="ExternalInput")
    u_out = nc.dram_tensor("u", [T, 2 * BPC], mybir.dt.float32,
                           kind="ExternalOutput")
    s_out = nc.dram_tensor("s", [1, NREN * BPC], mybir.dt.float32,
                           kind="ExternalOutput")

    with tile.TileContext(nc) as tc:
        with (
            tc.tile_pool(name="consts", bufs=1) as consts,
            tc.tile_pool(name="fpool", bufs=1) as fpool,
            tc.tile_pool(name="rawpool", bufs=3) as rawpool,
            tc.tile_pool(name="pipool", bufs=4) as pipool,
            tc.tile_pool(name="cipool", bufs=4) as cipool,
            tc.tile_pool(name="mmpsA", bufs=2, space="PSUM") as mmpsA,
            tc.tile_pool(name="mmpsB", bufs=2, space="PSUM") as mmpsB,
            tc.tile_pool(name="spsum", bufs=2, space="PSUM") as spsum,
            tc.tile_pool(name="rpool", bufs=2) as rpool,
        ):
            # ---- emissions F: 8 chunk tiles [T, BPC, 128] f32 ----
            F = []
            for c in range(NCH):
                fc = fpool.tile([T, BPC, 128], mybir.dt.float32, tag=f"F{c}",
                                name=f"F{c}")
                F.append(fc)

            def fcol(t):
                return F[t // 128][:, :, t % 128]

            raw0 = rawpool.tile([T, BPC, 128], mybir.dt.float32, tag="raw",
                                name="raw0")
            raw7 = rawpool.tile([T, BPC, 128], mybir.dt.float32, tag="raw",
                                name="raw7")

            def dma_piece(rawt, c, lo, hi):
                nc.sync.dma_start(out=rawt[:, :, lo:hi],
                                  in_=lg[:, :, c * 128 + lo:c * 128 + hi])

            def exp_piece(rawt, c, lo, hi):
                nc.scalar.activation(
                    out=F[c][:, :, lo:hi], in_=rawt[:, :, lo:hi],
                    func=mybir.ActivationFunctionType.Exp,
                )

            # fine-sliced first pieces so both chains launch early; the DMA
            # emission order below is the HWDGE grant order
            dma_piece(raw0, 0, 0, 16)
            eh_t = consts.tile([T, T], mybir.dt.float32)
            nc.sync.dma_start(out=eh_t[:], in_=eh[:, :])
            dma_piece(raw7, 7, 112, 128)
            ehT_t = consts.tile([T, T], mybir.dt.float32)
            nc.sync.dma_start(out=ehT_t[:], in_=ehT[:, :])
            exp_piece(raw0, 0, 0, 16)
            exp_piece(raw7, 7, 112, 128)
            dma_piece(raw0, 0, 16, 64)
            dma_piece(raw7, 7, 64, 112)
            exp_piece(raw0, 0, 16, 64)
            exp_piece(raw7, 7, 64, 112)
            dma_piece(raw0, 0, 64, 128)
            dma_piece(raw7, 7, 0, 64)
            exp_piece(raw0, 0, 64, 128)
            exp_piece(raw7, 7, 0, 64)

            ones_t = consts.tile([T, 1], mybir.dt.float32)
            nc.vector.memset(ones_t[:], 1.0)
            sacc = consts.tile([1, NREN * BPC], mybir.dt.float32)

            for c in (1, 6, 2, 5, 3, 4):
                rawc = rawpool.tile([T, BPC, 128], mybir.dt.float32, tag="raw",
                                    name=f"raw{c}")
                nc.sync.dma_start(out=rawc[:],
                                  in_=lg[:, :, c * 128:(c + 1) * 128])
                halves = (0, 1) if c < 4 else (1, 0)
                for h in halves:
                    exp_piece(rawc, c, h * 64, (h + 1) * 64)

            def renorm(v_ap, slot, fold_t):
                sp = spsum.tile([1, BPC], mybir.dt.float32, tag="sp", name="sp")
                nc.tensor.matmul(sp[:], ones_t[:], v_ap)
                nc.scalar.activation(
                    out=sacc[:, slot * BPC:(slot + 1) * BPC], in_=sp[:],
                    func=mybir.ActivationFunctionType.Copy,
                )
                rec = rpool.tile([1, BPC], mybir.dt.float32, tag="rec",
                                 name="rec")
                nc.vector.reciprocal(out=rec[:], in_=sp[:])
                rb = rpool.tile([T, BPC], mybir.dt.float32, tag="rb", name="rb")
                nc.gpsimd.partition_broadcast(rb[:], rec[:])
                # fold 1/S into a future F column, off the critical path
                nc.gpsimd.tensor_tensor(out=fcol(fold_t), in0=fcol(fold_t),
                                        in1=rb[:], op=mybir.AluOpType.mult)

            # final pi_MID / c_{MID+1} land in one shared tile -> one DMA;
            # the host does the tiny meet product pi^T Ehat c
            uend = consts.tile([T, 2 * BPC], mybir.dt.float32)

            # ---- bidirectional recurrence, interleaved emission ----
            pi_ap = fcol(0)       # pi_0 = exp(lg_0 + start)
            ci_ap = fcol(L - 1)   # c_{L-1} = exp(lg_{L-1} + end)
            for k in range(NSTEP_F):
                tf = k + 1
                psf = mmpsA.tile([T, BPC], mybir.dt.float32, tag="psf",
                                 name="psf")
                nc.tensor.matmul(psf[:], eh_t[:], pi_ap)
                if k < NSTEP_B:
                    tb = L - 2 - k
                    psb = mmpsB.tile([T, BPC], mybir.dt.float32, tag="psb",
                                     name="psb")
                    nc.tensor.matmul(psb[:], ehT_t[:], ci_ap)
                npi = (uend[:, 0:BPC] if k == NSTEP_F - 1 else
                       pipool.tile([T, BPC], mybir.dt.float32, tag="pi",
                                   name="pi")[:])
                nc.vector.tensor_tensor(out=npi, in0=psf[:], in1=fcol(tf),
                                        op=mybir.AluOpType.mult)
                pi_ap = npi
                if k < NSTEP_B:
                    nci = (uend[:, BPC:2 * BPC] if k == NSTEP_B - 1 else
                           cipool.tile([T, BPC], mybir.dt.float32, tag="ci",
                                       name="ci")[:])
                    nc.vector.tensor_tensor(out=nci, in0=psb[:],
                                            in1=fcol(tb),
                                            op=mybir.AluOpType.mult)
                    ci_ap = nci

                if tf in FWD_REN:
                    renorm(pi_ap, FWD_REN.index(tf), tf + APPLY_DELAY)
                if k < NSTEP_B and tb in BWD_REN:
                    renorm(ci_ap, 4 + BWD_REN.index(tb), tb - APPLY_DELAY)

            nc.sync.dma_start(out=s_out[:, :], in_=sacc[:])
            nc.sync.dma_start(out=u_out[:, :], in_=uend[:])

    nc.compile()
    return nc


_NC_CACHE = None


def _get_nc():
    global _NC_CACHE
    if _NC_CACHE is None:
        _NC_CACHE = _build()
    return _NC_CACHE


def kernel(inputs, tags, mask, transitions, start_transitions, end_transitions):
    logits = np.ascontiguousarray(inputs, dtype=np.float32)
    trans = np.asarray(transitions, dtype=np.float32)
    start_t = np.asarray(start_transitions, dtype=np.float32)
    end_t = np.asarray(end_transitions, dtype=np.float32)
    tags_i = np.asarray(tags).astype(np.int64, copy=False)
    maskf = np.asarray(mask).astype(np.float64)

    # ---------- device part: log-partition via bidirectional scaled pass ----
    lg = logits.copy()
    lg[:, 0, :] += start_t[None, :]
    lg[:, -1, :] += end_t[None, :]
    E = np.exp(trans.astype(np.float64))
    ghat = float(np.log(T * E.mean()))
    eh = (E * np.exp(-ghat)).astype(np.float32)
    ehT = np.ascontiguousarray(eh.T)
    # [NCORES, T, BPC, L]: tag-major per core so device DMAs need no transpose
    lgT = np.ascontiguousarray(
        lg.reshape(NCORES, BPC, L, T).transpose(0, 3, 1, 2))

    nc = _get_nc()
    in_maps = []
    for c in range(NCORES):
        in_maps.append({
            "lg": lgT[c],
            "eh": eh,
            "ehT": ehT,
        })
    res = run_bass_kernel_spmd(nc, in_maps, core_ids=list(range(NCORES)))

    u = np.stack([res.results[c]["u"] for c in range(NCORES)])     # (8, T, 2*BPC)
    s = np.stack([res.results[c]["s"] for c in range(NCORES)])     # (8, 1, NREN*BPC)
    pi_end = u[:, :, :BPC].astype(np.float64)                      # (8, T, BPC)
    ci_end = u[:, :, BPC:].astype(np.float64)                      # (8, T, BPC)
    # meet: w[b] = pi_MID^T Ehat c_{MID+1}
    w = np.einsum("cjb,jk,ckb->cb", pi_end, eh.astype(np.float64), ci_end)
    logZ = np.log(w.reshape(NCORES * BPC))
    srs = s.reshape(NCORES, NREN, BPC).astype(np.float64)
    logZ += np.log(srs).sum(axis=1).reshape(-1)
    logZ += (L - 1) * ghat

    # ---------- host part: gold-path numerator (tiny gathers) ----------
    lf64 = logits.astype(np.float64)
    emit = np.take_along_axis(lf64, tags_i[..., None], axis=2)[..., 0]   # (B, L)
    trans_sc = trans.astype(np.float64)[tags_i[:, :-1], tags_i[:, 1:]]   # (B, L-1)
    score = start_t.astype(np.float64)[tags_i[:, 0]]
    score = score + (trans_sc * maskf[:, 1:]).sum(axis=1)
    score = score + (emit[:, :-1] * maskf[:, :-1]).sum(axis=1)
    last_idx = maskf.astype(np.int64).sum(axis=1) - 1
    last_tags = np.take_along_axis(tags_i, last_idx[:, None], axis=1)[:, 0]
    last_input_score = lf64[np.arange(B), -1, last_tags]
    score = score + end_t.astype(np.float64)[last_tags] + last_input_score * maskf[:, -1]

    return np.float32(np.sum(score - logZ))


# revision 8
# speedup vs baseline: 1.0043x; 1.0007x over previous
"""Trainium2 Bass kernel for nn_ConditionalRandomField_52913997087452.

Computes sum_b [ gold_path_score(b) - log Z(b) ] for a linear-chain CRF with
B=128, L=1024, T=128, mask all-ones.

Strategy (data-parallel over batch, 16 per core x 8 cores), bidirectional:
  - The per-core serial bottleneck is the alpha recurrence's cross-engine
    latency (PE matmul visibility + DVE PSUM-read multiply), ~535 ns/step.
    Instead of one 1023-step forward chain, run TWO independent chains
    concurrently and meet in the middle:
        forward:  pi_t = f_t * (Ehat^T pi_{t-1}),  t = 1..MID
        backward: c_t  = f_t * (Ehat   c_{t+1}),   t = 1022..MID+1
    with Ehat = exp(transitions - ghat) and f_t = exp(lg_t) where lg is
    host-preprocessed: start/end transitions folded into t=0 / t=L-1, and
    every (b, t) column shifted by its log-sum-exp over tags (minus log T).
    That LSE shift keeps the per-step growth of pi/c at ~1.0, so NO on-device
    renormalization is needed; the host adds the exact shifts back in f64.
    Per batch column
        Z * e^{-(L-1) ghat - sum_t lse_t} = sum_jk pi_MID[j] Ehat[j,k] c_{MID+1}[k].
  - Emissions F are DMA'd in a host-pretransposed [T, B, L] layout (no PE
    transposes on device) and exponentiated by the Act engine. The first
    pieces of chunks 0 and 7 are sliced fine so both chains start early.
  - The two final chain vectors land in one shared tile -> one DMA; the host
    does the tiny meet product pi^T Ehat c and the final log.
  - The gold-path numerator is a tiny gather-and-sum done on the host.

The kernel builder is cached at module level so repeated kernel() calls
reuse the compiled program.
"""
import sys

if "/opt/trn_rl_repo" not in sys.path:
    sys.path.insert(0, "/opt/trn_rl_repo")

import numpy as np

import concourse.bacc as bacc
import concourse.tile as tile
from concourse import mybir
from concourse.bass_utils import run_bass_kernel_spmd

B = 128
L = 1024
T = 128
NCORES = 8
BPC = B // NCORES       # batch per core
NCH = L // 128          # 128-column F chunks
MID = 512               # fwd produces pi_1..pi_MID; bwd produces c_1022..c_{MID+1}
NSTEP_F = MID           # 512 fwd multiply steps
NSTEP_B = L - 2 - MID   # 510 bwd multiply steps


def _build():
    nc = bacc.Bacc("TRN2", target_bir_lowering=False)
    # host-pretransposed, column-LSE-normalized emissions: [tag, batch, time]
    lg = nc.dram_tensor("lg", [T, BPC, L], mybir.dt.float32, kind="ExternalInput")
    eh = nc.dram_tensor("eh", [T, T], mybir.dt.float32, kind="ExternalInput")
    ehT = nc.dram_tensor("ehT", [T, T], mybir.dt.float32, kind="ExternalInput")
    u_out = nc.dram_tensor("u", [T, 2 * BPC], mybir.dt.float32,
                           kind="ExternalOutput")

    with tile.TileContext(nc) as tc:
        with (
            tc.tile_pool(name="consts", bufs=1) as consts,
            tc.tile_pool(name="fpool", bufs=1) as fpool,
            tc.tile_pool(name="rawpool", bufs=3) as rawpool,
            tc.tile_pool(name="pipool", bufs=4) as pipool,
            tc.tile_pool(name="cipool", bufs=4) as cipool,
            tc.tile_pool(name="mmpsA", bufs=2, space="PSUM") as mmpsA,
            tc.tile_pool(name="mmpsB", bufs=2, space="PSUM") as mmpsB,
        ):
            # ---- emissions F: 8 chunk tiles [T, BPC, 128] f32 ----
            F = []
            for c in range(NCH):
                fc = fpool.tile([T, BPC, 128], mybir.dt.float32, tag=f"F{c}",
                                name=f"F{c}")
                F.append(fc)

            def fcol(t):
                return F[t // 128][:, :, t % 128]

            raw0 = rawpool.tile([T, BPC, 128], mybir.dt.float32, tag="raw",
                                name="raw0")
            raw7 = rawpool.tile([T, BPC, 128], mybir.dt.float32, tag="raw",
                                name="raw7")

            def dma_piece(rawt, c, lo, hi):
                nc.sync.dma_start(out=rawt[:, :, lo:hi],
                                  in_=lg[:, :, c * 128 + lo:c * 128 + hi])

            def exp_piece(rawt, c, lo, hi):
                nc.scalar.activation(
                    out=F[c][:, :, lo:hi], in_=rawt[:, :, lo:hi],
                    func=mybir.ActivationFunctionType.Exp,
                )

            # fine-sliced first pieces so both chains launch early; the DMA
            # emission order below is the HWDGE grant order
            dma_piece(raw0, 0, 0, 16)
            eh_t = consts.tile([T, T], mybir.dt.float32)
            nc.sync.dma_start(out=eh_t[:], in_=eh[:, :])
            dma_piece(raw7, 7, 112, 128)
            ehT_t = consts.tile([T, T], mybir.dt.float32)
            nc.sync.dma_start(out=ehT_t[:], in_=ehT[:, :])
            exp_piece(raw0, 0, 0, 16)
            exp_piece(raw7, 7, 112, 128)
            dma_piece(raw0, 0, 16, 64)
            dma_piece(raw7, 7, 64, 112)
            exp_piece(raw0, 0, 16, 64)
            exp_piece(raw7, 7, 64, 112)
            dma_piece(raw0, 0, 64, 128)
            dma_piece(raw7, 7, 0, 64)
            exp_piece(raw0, 0, 64, 128)
            exp_piece(raw7, 7, 0, 64)

            for c in (1, 6, 2, 5, 3, 4):
                rawc = rawpool.tile([T, BPC, 128], mybir.dt.float32, tag="raw",
                                    name=f"raw{c}")
                nc.sync.dma_start(out=rawc[:],
                                  in_=lg[:, :, c * 128:(c + 1) * 128])
                halves = (0, 1) if c < 4 else (1, 0)
                for h in halves:
                    exp_piece(rawc, c, h * 64, (h + 1) * 64)

            # final pi_MID / c_{MID+1} land in one shared tile -> one DMA;
            # the host does the tiny meet product pi^T Ehat c
            uend = consts.tile([T, 2 * BPC], mybir.dt.float32)

            # ---- bidirectional recurrence, interleaved emission ----
            pi_ap = fcol(0)       # pi_0 = exp(lg_0)  (start folded on host)
            ci_ap = fcol(L - 1)   # c_{L-1} = exp(lg_{L-1})  (end folded)
            for k in range(NSTEP_F):
                tf = k + 1
                psf = mmpsA.tile([T, BPC], mybir.dt.float32, tag="psf",
                                 name="psf")
                nc.tensor.matmul(psf[:], eh_t[:], pi_ap)
                if k < NSTEP_B:
                    tb = L - 2 - k
                    psb = mmpsB.tile([T, BPC], mybir.dt.float32, tag="psb",
                                     name="psb")
                    nc.tensor.matmul(psb[:], ehT_t[:], ci_ap)
                npi = (uend[:, 0:BPC] if k == NSTEP_F - 1 else
                       pipool.tile([T, BPC], mybir.dt.float32, tag="pi",
                                   name="pi")[:])
                nc.vector.tensor_tensor(out=npi, in0=psf[:], in1=fcol(tf),
                                        op=mybir.AluOpType.mult)
                pi_ap = npi
                if k < NSTEP_B:
                    nci = (uend[:, BPC:2 * BPC] if k == NSTEP_B - 1 else
                           cipool.tile([T, BPC], mybir.dt.float32, tag="ci",
                                       name="ci")[:])
                    nc.vector.tensor_tensor(out=nci, in0=psb[:],
                                            in1=fcol(tb),
                                            op=mybir.AluOpType.mult)
                    ci_ap = nci

            nc.sync.dma_start(out=u_out[:, :], in_=uend[:])

    nc.compile()
    return nc


_NC_CACHE = None


def _get_nc():
    global _NC_CACHE
    if _NC_CACHE is None:
        _NC_CACHE = _build()
    return _NC_CACHE


def kernel(inputs, tags, mask, transitions, start_transitions, end_transitions):
    logits = np.ascontiguousarray(inputs, dtype=np.float32)
    trans = np.asarray(transitions, dtype=np.float32)
    start_t = np.asarray(start_transitions, dtype=np.float32)
    end_t = np.asarray(end_transitions, dtype=np.float32)
    tags_i = np.asarray(tags).astype(np.int64, copy=False)
    maskf = np.asarray(mask).astype(np.float64)

    # ---------- device part: log-partition via bidirectional scaled pass ----
    lg = logits.copy()
    lg[:, 0, :] += start_t[None, :]
    lg[:, -1, :] += end_t[None, :]
    # per-(b, t) LSE shift: keeps on-device pi/c growth ~1.0 (no renorm)
    m = lg.max(axis=2)
    lse = m + np.log(
        np.exp(lg - m[:, :, None]).sum(axis=2, dtype=np.float64)
    ).astype(np.float32)                       # (B, L)
    lg -= (lse - np.float32(np.log(T)))[:, :, None]
    E = np.exp(trans.astype(np.float64))
    ghat = float(np.log(T * E.mean()))
    eh = (E * np.exp(-ghat)).astype(np.float32)
    ehT = np.ascontiguousarray(eh.T)
    # [NCORES, T, BPC, L]: tag-major per core so device DMAs need no transpose
    lgT = np.ascontiguousarray(
        lg.reshape(NCORES, BPC, L, T).transpose(0, 3, 1, 2))

    nc = _get_nc()
    in_maps = []
    for c in range(NCORES):
        in_maps.append({
            "lg": lgT[c],
            "eh": eh,
            "ehT": ehT,
        })
    res = run_bass_kernel_spmd(nc, in_maps, core_ids=list(range(NCORES)))

    u = np.stack([res.results[c]["u"] for c in range(NCORES)])     # (8, T, 2*BPC)
    pi_end = u[:, :, :BPC].astype(np.float64)                      # (8, T, BPC)
    ci_end = u[:, :, BPC:].astype(np.float64)                      # (8, T, BPC)
    # meet: w[b] = pi_MID^T Ehat c_{MID+1}
    w = np.einsum("cjb,jk,ckb->cb", pi_end, eh.astype(np.float64), ci_end)
    logZ = np.log(w.reshape(NCORES * BPC))
    logZ += (lse.astype(np.float64) - np.log(T)).sum(axis=1)
    logZ += (L - 1) * ghat

    # ---------- host part: gold-path numerator (tiny gathers) ----------
    lf64 = logits.astype(np.float64)
    emit = np.take_along_axis(lf64, tags_i[..., None], axis=2)[..., 0]   # (B, L)
    trans_sc = trans.astype(np.float64)[tags_i[:, :-1], tags_i[:, 1:]]   # (B, L-1)
    score = start_t.astype(np.float64)[tags_i[:, 0]]
    score = score + (trans_sc * maskf[:, 1:]).sum(axis=1)
    score = score + (emit[:, :-1] * maskf[:, :-1]).sum(axis=1)
    last_idx = maskf.astype(np.int64).sum(axis=1) - 1
    last_tags = np.take_along_axis(tags_i, last_idx[:, None], axis=1)[:, 0]
    last_input_score = lf64[np.arange(B), -1, last_tags]
    score = score + end_t.astype(np.float64)[last_tags] + last_input_score * maskf[:, -1]

    return np.float32(np.sum(score - logZ))


# revision 10
# speedup vs baseline: 1.0069x; 1.0026x over previous
"""Trainium2 Bass kernel for nn_ConditionalRandomField_52913997087452.

Computes sum_b [ gold_path_score(b) - log Z(b) ] for a linear-chain CRF with
B=128, L=1024, T=128, mask all-ones.

Strategy (data-parallel over batch, 16 per core x 8 cores), bidirectional:
  - The per-core serial bottleneck is the alpha recurrence's cross-engine
    latency (PE matmul visibility + DVE PSUM-read multiply), ~535 ns/step.
    Instead of one 1023-step forward chain, run TWO independent chains
    concurrently and meet in the middle:
        forward:  pi_t = f_t * (Ehat^T pi_{t-1}),  t = 1..MID
        backward: c_t  = f_t * (Ehat   c_{t+1}),   t = 1022..MID+1
    with Ehat = exp(transitions - ghat) and f_t = exp(lg_t) where lg is
    host-preprocessed: start/end transitions folded into t=0 / t=L-1, and
    every (b, t) column shifted by its log-sum-exp over tags (minus log T).
    That LSE shift keeps the per-step growth of pi/c at ~1.0, so NO on-device
    renormalization is needed; the host adds the exact shifts back in f64.
    Per batch column
        Z * e^{-(L-1) ghat - sum_t lse_t} = sum_jk pi_MID[j] Ehat[j,k] c_{MID+1}[k].
  - Emissions F are DMA'd in a host-pretransposed [T, B, L] layout (no PE
    transposes on device) and exponentiated by the Act engine. The first
    pieces of chunks 0 and 7 are sliced fine so both chains start early.
  - The two final chain vectors land in one shared tile -> one DMA; the host
    does the tiny meet product pi^T Ehat c and the final log.
  - The gold-path numerator is a tiny gather-and-sum done on the host.

The kernel builder is cached at module level so repeated kernel() calls
reuse the compiled program.
"""
import sys

if "/opt/trn_rl_repo" not in sys.path:
    sys.path.insert(0, "/opt/trn_rl_repo")

import numpy as np

import concourse.bacc as bacc
import concourse.tile as tile
from concourse import mybir
from concourse.bass_utils import run_bass_kernel_spmd

B = 128
L = 1024
T = 128
NCORES = 8
BPC = B // NCORES       # batch per core
NCH = L // 128          # 128-column F chunks
MID = 512               # fwd produces pi_1..pi_MID; bwd produces c_1022..c_{MID+1}
NSTEP_F = MID           # 512 fwd multiply steps
NSTEP_B = L - 2 - MID   # 510 bwd multiply steps


def _build():
    nc = bacc.Bacc("TRN2", target_bir_lowering=False)
    # host-pretransposed, column-LSE-normalized emissions: [tag, batch, time]
    lg = nc.dram_tensor("lg", [T, BPC, L], mybir.dt.float32, kind="ExternalInput")
    eh = nc.dram_tensor("eh", [T, T], mybir.dt.float32, kind="ExternalInput")
    ehT = nc.dram_tensor("ehT", [T, T], mybir.dt.float32, kind="ExternalInput")
    u_out = nc.dram_tensor("u", [T, 2 * BPC], mybir.dt.float32,
                           kind="ExternalOutput")

    with tile.TileContext(nc) as tc:
        with (
            tc.tile_pool(name="consts", bufs=1) as consts,
            tc.tile_pool(name="fpool", bufs=1) as fpool,
            tc.tile_pool(name="rawpool", bufs=3) as rawpool,
            tc.tile_pool(name="pipool", bufs=4) as pipool,
            tc.tile_pool(name="cipool", bufs=4) as cipool,
            tc.tile_pool(name="mmpsA", bufs=2, space="PSUM") as mmpsA,
            tc.tile_pool(name="mmpsB", bufs=2, space="PSUM") as mmpsB,
        ):
            # ---- emissions F: 8 chunk tiles [T, BPC, 128] f32 ----
            F = []
            for c in range(NCH):
                fc = fpool.tile([T, BPC, 128], mybir.dt.float32, tag=f"F{c}",
                                name=f"F{c}")
                F.append(fc)

            def fcol(t):
                return F[t // 128][:, :, t % 128]

            raw0 = rawpool.tile([T, BPC, 128], mybir.dt.float32, tag="raw",
                                name="raw0")
            raw7 = rawpool.tile([T, BPC, 128], mybir.dt.float32, tag="raw",
                                name="raw7")

            def dma_piece(rawt, c, lo, hi):
                nc.sync.dma_start(out=rawt[:, :, lo:hi],
                                  in_=lg[:, :, c * 128 + lo:c * 128 + hi])

            def exp_piece(rawt, c, lo, hi):
                nc.scalar.activation(
                    out=F[c][:, :, lo:hi], in_=rawt[:, :, lo:hi],
                    func=mybir.ActivationFunctionType.Exp,
                )

            # fine-sliced first pieces so both chains launch early; the DMA
            # emission order below is the HWDGE grant order
            dma_piece(raw0, 0, 0, 16)
            eh_t = consts.tile([T, T], mybir.dt.float32)
            nc.sync.dma_start(out=eh_t[:], in_=eh[:, :])
            dma_piece(raw7, 7, 112, 128)
            ehT_t = consts.tile([T, T], mybir.dt.float32)
            nc.sync.dma_start(out=ehT_t[:], in_=ehT[:, :])
            exp_piece(raw0, 0, 0, 16)
            exp_piece(raw7, 7, 112, 128)
            dma_piece(raw0, 0, 16, 64)
            dma_piece(raw7, 7, 64, 112)
            exp_piece(raw0, 0, 16, 64)
            exp_piece(raw7, 7, 64, 112)
            dma_piece(raw0, 0, 64, 128)
            dma_piece(raw7, 7, 0, 64)
            exp_piece(raw0, 0, 64, 128)
            exp_piece(raw7, 7, 0, 64)

            for c in (1, 6, 2, 5, 3, 4):
                rawc = rawpool.tile([T, BPC, 128], mybir.dt.float32, tag="raw",
                                    name=f"raw{c}")
                nc.sync.dma_start(out=rawc[:],
                                  in_=lg[:, :, c * 128:(c + 1) * 128])
                halves = (0, 1) if c < 4 else (1, 0)
                for h in halves:
                    exp_piece(rawc, c, h * 64, (h + 1) * 64)

            # final pi_MID / c_{MID+1} land in one shared tile -> one DMA;
            # the host does the tiny meet product pi^T Ehat c
            uend = consts.tile([T, 2 * BPC], mybir.dt.float32)

            # A multiply whose F column starts a new exp piece would carry a
            # third sem wait (Act), pushing its PE wait into a SEQ-blocking
            # EventSemaphore (+~100ns on the chain). A tiny DVE read of the
            # piece a few steps early carries the Act wait instead, so the
            # chain multiply's wait is elided as redundant.
            scratch = consts.tile([1, BPC], mybir.dt.float32)
            # k -> (fwd piece-start col, bwd piece-start col) to prefetch
            PREFETCH = {13: (16, 1007)}
            for ts in (64, 128, 192, 256, 320, 384, 448):
                PREFETCH[ts - 9] = (ts, 1022 - (ts - 9) - 8)

            def prefetch(t):
                nc.vector.tensor_copy(out=scratch[:],
                                      in_=F[t // 128][0:1, :, t % 128])

            # ---- bidirectional recurrence, interleaved emission ----
            pi_ap = fcol(0)       # pi_0 = exp(lg_0)  (start folded on host)
            ci_ap = fcol(L - 1)   # c_{L-1} = exp(lg_{L-1})  (end folded)
            for k in range(NSTEP_F):
                tf = k + 1
                psf = mmpsA.tile([T, BPC], mybir.dt.float32, tag="psf",
                                 name="psf")
                nc.tensor.matmul(psf[:], eh_t[:], pi_ap)
                if k < NSTEP_B:
                    tb = L - 2 - k
                    psb = mmpsB.tile([T, BPC], mybir.dt.float32, tag="psb",
                                     name="psb")
                    nc.tensor.matmul(psb[:], ehT_t[:], ci_ap)
                npi = (uend[:, 0:BPC] if k == NSTEP_F - 1 else
                       pipool.tile([T, BPC], mybir.dt.float32, tag="pi",
                                   name="pi")[:])
                nc.vector.tensor_tensor(out=npi, in0=psf[:], in1=fcol(tf),
                                        op=mybir.AluOpType.mult)
                pi_ap = npi
                if k < NSTEP_B:
                    nci = (uend[:, BPC:2 * BPC] if k == NSTEP_B - 1 else
                           cipool.tile([T, BPC], mybir.dt.float32, tag="ci",
                                       name="ci")[:])
                    nc.vector.tensor_tensor(out=nci, in0=psb[:],
                                            in1=fcol(tb),
                                            op=mybir.AluOpType.mult)
                    ci_ap = nci

                if k in PREFETCH:
                    tsf, tsb = PREFETCH[k]
                    prefetch(tsf)
                    prefetch(tsb)

            nc.sync.dma_start(out=u_out[:, :], in_=uend[:])

    nc.compile()
    return nc


_NC_CACHE = None


def _get_nc():
    global _NC_CACHE
    if _NC_CACHE is None:
        _NC_CACHE = _build()
    return _NC_CACHE


def kernel(inputs, tags, mask, transitions, start_transitions, end_transitions):
    logits = np.ascontiguousarray(inputs, dtype=np.float32)
    trans = np.asarray(transitions, dtype=np.float32)
    start_t = np.asarray(start_transitions, dtype=np.float32)
    end_t = np.asarray(end_transitions, dtype=np.float32)
    tags_i = np.asarray(tags).astype(np.int64, copy=False)
    maskf = np.asarray(mask).astype(np.float64)

    # ---------- device part: log-partition via bidirectional scaled pass ----
    lg = logits.copy()
    lg[:, 0, :] += start_t[None, :]
    lg[:, -1, :] += end_t[None, :]
    # per-(b, t) LSE shift: keeps on-device pi/c growth ~1.0 (no renorm)
    m = lg.max(axis=2)
    lse = m + np.log(
        np.exp(lg - m[:, :, None]).sum(axis=2, dtype=np.float64)
    ).astype(np.float32)                       # (B, L)
    lg -= (lse - np.float32(np.log(T)))[:, :, None]
    E = np.exp(trans.astype(np.float64))
    ghat = float(np.log(T * E.mean()))
    eh = (E * np.exp(-ghat)).astype(np.float32)
    ehT = np.ascontiguousarray(eh.T)
    # [NCORES, T, BPC, L]: tag-major per core so device DMAs need no transpose
    lgT = np.ascontiguousarray(
        lg.reshape(NCORES, BPC, L, T).transpose(0, 3, 1, 2))

    nc = _get_nc()
    in_maps = []
    for c in range(NCORES):
        in_maps.append({
            "lg": lgT[c],
            "eh": eh,
            "ehT": ehT,
        })
    res = run_bass_kernel_spmd(nc, in_maps, core_ids=list(range(NCORES)))

    u = np.stack([res.results[c]["u"] for c in range(NCORES)])     # (8, T, 2*BPC)
    pi_end = u[:, :, :BPC].astype(np.float64)                      # (8, T, BPC)
    ci_end = u[:, :, BPC:].astype(np.float64)                      # (8, T, BPC)
    # meet: w[b] = pi_MID^T Ehat c_{MID+1}
    w = np.einsum("cjb,jk,ckb->cb", pi_end, eh.astype(np.float64), ci_end)
    logZ = np.log(w.reshape(NCORES * BPC))
    logZ += (lse.astype(np.float64) - np.log(T)).sum(axis=1)
    logZ += (L - 1) * ghat

    # ---------- host part: gold-path numerator (tiny gathers) ----------
    lf64 = logits.astype(np.float64)
    emit = np.take_along_axis(lf64, tags_i[..., None], axis=2)[..., 0]   # (B, L)
    trans_sc = trans.astype(np.float64)[tags_i[:, :-1], tags_i[:, 1:]]   # (B, L-1)
    score = start_t.astype(np.float64)[tags_i[:, 0]]
    score = score + (trans_sc * maskf[:, 1:]).sum(axis=1)
    score = score + (emit[:, :-1] * maskf[:, :-1]).sum(axis=1)
    last_idx = maskf.astype(np.int64).sum(axis=1) - 1
    last_tags = np.take_along_axis(tags_i, last_idx[:, None], axis=1)[:, 0]
    last_input_score = lf64[np.arange(B), -1, last_tags]
    score = score + end_t.astype(np.float64)[last_tags] + last_input_score * maskf[:, -1]

    return np.float32(np.sum(score - logZ))


# revision 19
# speedup vs baseline: 1.0113x; 1.0044x over previous
"""Trainium2 Bass kernel for nn_ConditionalRandomField_52913997087452.

Computes sum_b [ gold_path_score(b) - log Z(b) ] for a linear-chain CRF with
B=128, L=1024, T=128, mask all-ones.

Strategy (data-parallel over batch, 16 per core x 8 cores), bidirectional:
  - The per-core serial bottleneck is the alpha recurrence's cross-engine
    latency (PE matmul visibility + DVE PSUM-read multiply), ~535 ns/step.
    Instead of one 1023-step forward chain, run TWO independent chains
    concurrently and meet in the middle:
        forward:  pi_t = f_t * (Ehat^T pi_{t-1}),  t = 1..MID
        backward: c_t  = f_t * (Ehat   c_{t+1}),   t = 1022..MID+1
    with Ehat = exp(transitions - ghat) and f_t = exp(lg_t) where lg is
    host-preprocessed: start/end transitions folded into t=0 / t=L-1, and
    every (b, t) column shifted by its log-sum-exp over tags (minus log T).
    That LSE shift keeps the per-step growth of pi/c at ~1.0, so NO on-device
    renormalization is needed; the host adds the exact shifts back in f64.
    Per batch column
        Z * e^{-(L-1) ghat - sum_t lse_t} = sum_jk pi_MID[j] Ehat[j,k] c_{MID+1}[k].
  - Emissions F are DMA'd in a host-pretransposed [T, B, L] layout (no PE
    transposes on device) and exponentiated by the Act engine. The first
    pieces of chunks 0 and 7 are sliced fine so both chains start early.
  - The two final chain vectors land in one shared tile -> one DMA; the host
    does the tiny meet product pi^T Ehat c and the final log.
  - The gold-path numerator is a tiny gather-and-sum done on the host.

The kernel builder is cached at module level so repeated kernel() calls
reuse the compiled program.
"""
import sys

if "/opt/trn_rl_repo" not in sys.path:
    sys.path.insert(0, "/opt/trn_rl_repo")

import numpy as np

import concourse.bacc as bacc
import concourse.tile as tile
from concourse import mybir
from concourse.bass_utils import run_bass_kernel_spmd

B = 128
L = 1024
T = 128
NCORES = 8
BPC = B // NCORES       # batch per core
NCH = L // 128          # 128-column F chunks
MID = 511               # fwd produces pi_1..pi_MID; bwd produces c_1022..c_{MID+1}
NSTEP_F = MID           # 511 fwd multiply steps
NSTEP_B = L - 2 - MID   # 511 bwd multiply steps


def _build():
    nc = bacc.Bacc("TRN2", target_bir_lowering=False)
    # host-pretransposed, column-LSE-normalized emissions: [tag, batch, time]
    lg = nc.dram_tensor("lg", [T, BPC, L], mybir.dt.float32, kind="ExternalInput")
    # chain heads packed contiguously: cols 0:16 of chunk 0 + 112:128 of
    # chunk 7 -> one 128-descriptor 2KB-per-partition DMA
    hd = nc.dram_tensor("hd", [T, BPC, 32], mybir.dt.float32,
                        kind="ExternalInput")
    eh = nc.dram_tensor("eh", [T, T], mybir.dt.float32, kind="ExternalInput")
    ehT = nc.dram_tensor("ehT", [T, T], mybir.dt.float32, kind="ExternalInput")
    u_out = nc.dram_tensor("u", [T, 2 * BPC], mybir.dt.float32,
                           kind="ExternalOutput")

    with tile.TileContext(nc) as tc:
        with (
            tc.tile_pool(name="consts", bufs=1) as consts,
            tc.tile_pool(name="fpool", bufs=1) as fpool,
            tc.tile_pool(name="rawpool", bufs=3) as rawpool,
            tc.tile_pool(name="pipool", bufs=4) as pipool,
            tc.tile_pool(name="cipool", bufs=4) as cipool,
            tc.tile_pool(name="mmpsA", bufs=2, space="PSUM") as mmpsA,
            tc.tile_pool(name="mmpsB", bufs=2, space="PSUM") as mmpsB,
        ):
            # ---- emissions F: 8 chunk tiles [T, BPC, 128] f32 ----
            F = []
            for c in range(NCH):
                fc = fpool.tile([T, BPC, 128], mybir.dt.float32, tag=f"F{c}",
                                name=f"F{c}")
                F.append(fc)

            head = consts.tile([T, BPC, 32], mybir.dt.float32)

            def fcol(t):
                if t < 16:
                    return head[:, :, t]
                if t >= L - 16:
                    return head[:, :, 16 + t - (L - 16)]
                return F[t // 128][:, :, t % 128]

            raw0 = rawpool.tile([T, BPC, 128], mybir.dt.float32, tag="raw",
                                name="raw0")
            raw7 = rawpool.tile([T, BPC, 128], mybir.dt.float32, tag="raw",
                                name="raw7")

            def dma_piece(rawt, c, lo, hi):
                nc.sync.dma_start(out=rawt[:, :, lo:hi],
                                  in_=lg[:, :, c * 128 + lo:c * 128 + hi])

            def exp_piece(rawt, c, lo, hi):
                nc.scalar.activation(
                    out=F[c][:, :, lo:hi], in_=rawt[:, :, lo:hi],
                    func=mybir.ActivationFunctionType.Exp,
                )

            # chain-head DMA first (contiguous layout -> fast), then the
            # transition matrices; emission order below = HWDGE grant order
            headraw = consts.tile([T, BPC, 32], mybir.dt.float32)
            nc.sync.dma_start(out=headraw[:], in_=hd[:, :, :])
            eh_t = consts.tile([T, T], mybir.dt.float32)
            nc.sync.dma_start(out=eh_t[:], in_=eh[:, :])
            ehT_t = consts.tile([T, T], mybir.dt.float32)
            nc.sync.dma_start(out=ehT_t[:], in_=ehT[:, :])
            nc.scalar.activation(
                out=head[:, :, 0:16], in_=headraw[:, :, 0:16],
                func=mybir.ActivationFunctionType.Exp,
            )
            nc.scalar.activation(
                out=head[:, :, 16:32], in_=headraw[:, :, 16:32],
                func=mybir.ActivationFunctionType.Exp,
            )
            dma_piece(raw0, 0, 16, 64)
            dma_piece(raw7, 7, 64, 112)
            exp_piece(raw0, 0, 16, 64)
            exp_piece(raw7, 7, 64, 112)
            dma_piece(raw0, 0, 64, 128)
            dma_piece(raw7, 7, 0, 64)
            exp_piece(raw0, 0, 64, 128)
            exp_piece(raw7, 7, 0, 64)

            for c in (1, 6, 2, 5, 3, 4):
                rawc = rawpool.tile([T, BPC, 128], mybir.dt.float32, tag="raw",
                                    name=f"raw{c}")
                nc.sync.dma_start(out=rawc[:],
                                  in_=lg[:, :, c * 128:(c + 1) * 128])
                halves = (0, 1) if c < 4 else (1, 0)
                for h in halves:
                    exp_piece(rawc, c, h * 64, (h + 1) * 64)

            # final pi_MID / c_{MID+1} land in one shared tile -> one DMA;
            # the host does the tiny meet product pi^T Ehat c
            uend = consts.tile([T, 2 * BPC], mybir.dt.float32)

            # A multiply whose F column starts a new exp piece would carry a
            # third sem wait (Act), pushing its PE wait into a SEQ-blocking
            # EventSemaphore (+~100ns on the chain). A tiny DVE read of the
            # piece a few steps early carries the Act wait instead, so the
            # chain multiply's wait is elided as redundant.
            scratch = consts.tile([1, BPC], mybir.dt.float32)
            # k -> fwd / bwd piece-start col to prefetch (staggered so the
            # two copies never share one step's DVE slot)
            PF = {13: 16}
            PB = {14: 1007}
            for ts in (64, 128, 192, 256, 320, 384, 448):
                PF[ts - 9] = ts
                PB[ts - 8] = 1022 - (ts - 9) - 8

            def prefetch(t, anchor):
                # in1 anchors this copy to the current chain step so the
                # scheduler cannot hoist it before its exp's wait is live
                nc.vector.tensor_tensor(out=scratch[:],
                                        in0=F[t // 128][0:1, :, t % 128],
                                        in1=anchor[0:1, :],
                                        op=mybir.AluOpType.mult)

            # ---- bidirectional recurrence, interleaved emission ----
            pi_ap = fcol(0)       # pi_0 = exp(lg_0)  (start folded on host)
            ci_ap = fcol(L - 1)   # c_{L-1} = exp(lg_{L-1})  (end folded)
            for k in range(NSTEP_F):
                tf = k + 1
                psf = mmpsA.tile([T, BPC], mybir.dt.float32, tag="psf",
                                 name="psf")
                nc.tensor.matmul(psf[:], eh_t[:], pi_ap)
                if k < NSTEP_B:
                    tb = L - 2 - k
                    psb = mmpsB.tile([T, BPC], mybir.dt.float32, tag="psb",
                                     name="psb")
                    nc.tensor.matmul(psb[:], ehT_t[:], ci_ap)
                npi = (uend[:, 0:BPC] if k == NSTEP_F - 1 else
                       pipool.tile([T, BPC], mybir.dt.float32, tag="pi",
                                   name="pi")[:])
                nc.vector.tensor_tensor(out=npi, in0=psf[:], in1=fcol(tf),
                                        op=mybir.AluOpType.mult)
                pi_ap = npi
                if k < NSTEP_B:
                    nci = (uend[:, BPC:2 * BPC] if k == NSTEP_B - 1 else
                           cipool.tile([T, BPC], mybir.dt.float32, tag="ci",
                                       name="ci")[:])
                    nc.vector.tensor_tensor(out=nci, in0=psb[:],
                                            in1=fcol(tb),
                                            op=mybir.AluOpType.mult)
                    ci_ap = nci

                if k in PF:
                    prefetch(PF[k], pi_ap)
                if k in PB:
                    prefetch(PB[k], ci_ap)

            nc.sync.dma_start(out=u_out[:, :], in_=uend[:])

    nc.compile()
    return nc


_NC_CACHE = None


def _get_nc():
    global _NC_CACHE
    if _NC_CACHE is None:
        _NC_CACHE = _build()
    return _NC_CACHE


def kernel(inputs, tags, mask, transitions, start_transitions, end_transitions):
    logits = np.ascontiguousarray(inputs, dtype=np.float32)
    trans = np.asarray(transitions, dtype=np.float32)
    start_t = np.asarray(start_transitions, dtype=np.float32)
    end_t = np.asarray(end_transitions, dtype=np.float32)
    tags_i = np.asarray(tags).astype(np.int64, copy=False)
    maskf = np.asarray(mask).astype(np.float64)

    # ---------- device part: log-partition via bidirectional scaled pass ----
    lg = logits.copy()
    lg[:, 0, :] += start_t[None, :]
    lg[:, -1, :] += end_t[None, :]
    # per-(b, t) LSE shift: keeps on-device pi/c growth ~1.0 (no renorm)
    m = lg.max(axis=2)
    lse = m + np.log(
        np.exp(lg - m[:, :, None]).sum(axis=2, dtype=np.float64)
    ).astype(np.float32)                       # (B, L)
    lg -= (lse - np.float32(np.log(T)))[:, :, None]
    E = np.exp(trans.astype(np.float64))
    ghat = float(np.log(T * E.mean()))
    eh = (E * np.exp(-ghat)).astype(np.float32)
    ehT = np.ascontiguousarray(eh.T)
    # [NCORES, T, BPC, L]: tag-major per core so device DMAs need no transpose
    lgT = np.ascontiguousarray(
        lg.reshape(NCORES, BPC, L, T).transpose(0, 3, 1, 2))
    hdT = np.ascontiguousarray(
        np.concatenate([lgT[:, :, :, 0:16], lgT[:, :, :, L - 16:L]], axis=3))

    nc = _get_nc()
    in_maps = []
    for c in range(NCORES):
        in_maps.append({
            "lg": lgT[c],
            "hd": hdT[c],
            "eh": eh,
            "ehT": ehT,
        })
    res = run_bass_kernel_spmd(nc, in_maps, core_ids=list(range(NCORES)))

    u = np.stack([res.results[c]["u"] for c in range(NCORES)])     # (8, T, 2*BPC)
    pi_end = u[:, :, :BPC].astype(np.float64)                      # (8, T, BPC)
    ci_end = u[:, :, BPC:].astype(np.float64)                      # (8, T, BPC)
    # meet: w[b] = pi_MID^T Ehat c_{MID+1}
    w = np.einsum("cjb,jk,ckb->cb", pi_end, eh.astype(np.float64), ci_end)
    logZ = np.log(w.reshape(NCORES * BPC))
    logZ += (lse.astype(np.float64) - np.log(T)).sum(axis=1)
    logZ += (L - 1) * ghat

    # ---------- host part: gold-path numerator (tiny gathers) ----------
    lf64 = logits.astype(np.float64)
    emit = np.take_along_axis(lf64, tags_i[..., None], axis=2)[..., 0]   # (B, L)
    trans_sc = trans.astype(np.float64)[tags_i[:, :-1], tags_i[:, 1:]]   # (B, L-1)
    score = start_t.astype(np.float64)[tags_i[:, 0]]
    score = score + (trans_sc * maskf[:, 1:]).sum(axis=1)
    score = score + (emit[:, :-1] * maskf[:, :-1]).sum(axis=1)
    last_idx = maskf.astype(np.int64).sum(axis=1) - 1
    last_tags = np.take_along_axis(tags_i, last_idx[:, None], axis=1)[:, 0]
    last_input_score = lf64[np.arange(B), -1, last_tags]
    score = score + end_t.astype(np.float64)[last_tags] + last_input_score * maskf[:, -1]

    return np.float32(np.sum(score - logZ))
